# revision 1
# baseline (speedup 1.0000x reference)
"""MoE block (AdaptFormer adapters, top-2 of 8 experts) on 8 TRN2 NeuronCores.

Data-parallel over the 8192 tokens (1024/core), router + expert adapter
weights replicated. Per core:
  - x is shipped as an exact bf16 hi/lo split (xh + xl == x to 2^-17),
    pre-transposed on the host to [D, T] (pure layout prep, like the
    Wd/Wu repacking) so xT loads are plain contiguous DMAs.
  - logits = x @ w_gate exactly enough (error ~3e-6, far below the 3.6e-5
    min top-2/3 logit gap of this dataset) via three bf16 matmuls
    xh@wg_h + xh@wg_l + xl@wg_h with w_gate stationary -> lT [8, tok].
  - xT is rebuilt to float32r by DVE adds (xh + xl) and feeds
    hT = Wd_all^T-stationary matmuls in f32r: HT chunks [128-of-512, tok]
    land already transposed for the second matmul (no PE transposes).
  - gates: lT slices are PE-transposed back per 128-token tile, the top-2
    softmax (x0.5) runs on DVE/ACT, g2 is PE-transposed to g2T and
    expanded across the 512-wide expert axis by a 0/1 block matrix matmul
    (GB), then HG = relu(HT) * GB on ACT+DVE.
  - out tiles = HG-slices @ Wu_flat accumulated over the expert axis.
All experts computed densely; sparse gates zero the non-top-2 terms
(mathematically identical to dispatch/combine).
"""
import numpy as np
import ml_dtypes
from contextlib import ExitStack

import concourse.bass as bass
import concourse.tile as tile
from concourse.tile import add_dep_helper
from concourse import bacc, mybir
from concourse.bass_utils import run_bass_kernel_spmd

N_CORES = 8
B_DIM, S_DIM, D = 2, 4096, 1024
T = B_DIM * S_DIM          # 8192 tokens
TC = T // N_CORES          # 1024 tokens per core
E, BK = 8, 64              # experts, bottleneck
EB = E * BK                # 512 concatenated expert axis
P = 128
NTT = TC // P              # token tiles per core
KC = D // P                # D chunks
BC = EB // P               # bottleneck chunks
LBLK = 512                 # token block for the blocked phases
NLB = TC // LBLK
TPB = LBLK // P            # token tiles per block
SCALE = 0.5
N_WARM = 10                # PE warm-up matmuls during initial DMA wait

F32 = mybir.dt.float32
F32R = mybir.dt.float32r
BF16 = mybir.dt.bfloat16
AL = mybir.AluOpType
ACTF = mybir.ActivationFunctionType
AX = mybir.AxisListType

_BUILD_CACHE = {}


def _build(include_bd: bool, include_bu: bool, reps: int = 1):
    key = (include_bd, include_bu, reps)
    if key in _BUILD_CACHE:
        return _BUILD_CACHE[key]

    nc = bacc.Bacc("TRN2", target_bir_lowering=False, debug=False,
                   num_devices=N_CORES)
    # x halves, shipped pre-transposed: [D, TC] bf16
    xh_d = nc.dram_tensor("xh", [D, TC], BF16, kind="ExternalInput").ap()
    xl_d = nc.dram_tensor("xl", [D, TC], BF16, kind="ExternalInput").ap()
    wd_d = nc.dram_tensor("wd", [D, EB], F32, kind="ExternalInput").ap()
    wu_d = nc.dram_tensor("wu", [EB, D], F32, kind="ExternalInput").ap()
    wgh_d = nc.dram_tensor("wgh", [D, E], BF16, kind="ExternalInput").ap()
    wgl_d = nc.dram_tensor("wgl", [D, E], BF16, kind="ExternalInput").ap()
    id_d = nc.dram_tensor("ident", [P, P], F32, kind="ExternalInput").ap()
    eb_d = nc.dram_tensor("eblk", [E, EB], F32, kind="ExternalInput").ap()
    if include_bd:
        bd_d = nc.dram_tensor("bd", [P, BC], F32, kind="ExternalInput").ap()
    if include_bu:
        bu_d = nc.dram_tensor("bu", [E, D], F32, kind="ExternalInput").ap()
    out_d = nc.dram_tensor("out", [TC, D], F32, kind="ExternalOutput").ap()

    with tile.TileContext(nc) as tc, ExitStack() as ctx:
        wpool = ctx.enter_context(tc.tile_pool(name="weights", bufs=1))
        hgpool = ctx.enter_context(tc.tile_pool(name="hg", bufs=6))
        gpool = ctx.enter_context(tc.tile_pool(name="gates", bufs=2))
        opool = ctx.enter_context(tc.tile_pool(name="osb", bufs=3))

        ht_ps_pool = ctx.enter_context(
            tc.tile_pool(name="htps", bufs=3, space="PSUM"))
        gb_ps_pool = ctx.enter_context(
            tc.tile_pool(name="gbps", bufs=1, space="PSUM"))
        lt_ps_pool = ctx.enter_context(
            tc.tile_pool(name="ltps", bufs=1, space="PSUM"))
        lb_ps_pool = ctx.enter_context(
            tc.tile_pool(name="lbps", bufs=1, space="PSUM"))
        o_ps_pool = ctx.enter_context(
            tc.tile_pool(name="ops", bufs=2, space="PSUM"))

        # PE warm-up first: source tile is memset (no DMA wait), one long
        # accumulation group so the matmuls run back-to-back and trip the
        # HAM un-throttle within ~5us.
        warm32 = wpool.tile([P, EB], F32, tag="warm32")
        nc.vector.memset(warm32[:], 0.001)
        warm_src = wpool.tile([P, EB], F32R, tag="warmsrc")
        nc.vector.tensor_copy(warm_src[:], warm32[:])
        warm_ps = gb_ps_pool.tile([P, EB], F32R, tag="gbps")
        for i in range(N_WARM):
            nc.tensor.matmul(warm_ps[:].bitcast(F32), warm_src[:, 0:P],
                             warm_src[:], start=(i == 0),
                             stop=(i == N_WARM - 1))

        # x halves arrive pre-transposed: plain contiguous chunk loads,
        # with the f32r reconstruction add right behind each chunk pair.
        xht = [wpool.tile([P, TC], BF16, tag=f"xht{c}", name=f"xht{c}")
               for c in range(KC)]
        xlt = [wpool.tile([P, TC], BF16, tag=f"xlt{c}", name=f"xlt{c}")
               for c in range(KC)]
        xtr = [wpool.tile([P, TC], F32R, tag=f"xtr{c}", name=f"xtr{c}")
               for c in range(KC)]
        last_x = None
        for c in range(KC):
            nc.sync.dma_start(xht[c][:], xh_d[bass.ts(c, P), :])
            last_x = nc.sync.dma_start(xlt[c][:], xl_d[bass.ts(c, P), :])
            nc.vector.tensor_tensor(xtr[c][:], xht[c][:], xlt[c][:],
                                    op=AL.add)
            # HAM-warm fillers: keep the PE array busy while x streams in
            for _ in range(3):
                nc.tensor.matmul(warm_ps[:].bitcast(F32), warm_src[:, 0:P],
                                 warm_src[:], start=True, stop=True)

        # small constants (copy-mode DMAs, before any xbar transpose)
        ident = wpool.tile([P, P], F32, tag="ident")
        nc.sync.dma_start(ident[:], id_d)
        ident_r = wpool.tile([P, P], F32R, tag="identr")
        nc.sync.dma_start(ident_r[:], id_d.bitcast(F32R))
        eblk = wpool.tile([E, EB], F32R, tag="eblk")
        nc.sync.dma_start(eblk[:], eb_d.bitcast(F32R))
        wgh_sb = wpool.tile([P, KC, E], BF16, tag="wgh")
        nc.sync.dma_start(wgh_sb[:], wgh_d.rearrange("(c p) n -> p c n", p=P))
        wgl_sb = wpool.tile([P, KC, E], BF16, tag="wgl")
        nc.sync.dma_start(wgl_sb[:], wgl_d.rearrange("(c p) n -> p c n", p=P))

        def wdma(dst, src):
            i = nc.sync.dma_start(dst, src)
            add_dep_helper(i.ins, last_x.ins, sync=True,
                           reason="weights stream after x")
            return i

        wd_sb = [wpool.tile([P, EB], F32R, tag=f"wd{c}", name=f"wd{c}")
                 for c in range(KC)]
        for c in range(KC):
            wdma(wd_sb[c][:], wd_d.bitcast(F32R)[bass.ts(c, P), :])
        wu_sb = [wpool.tile([P, D], F32R, tag=f"wu{k}", name=f"wu{k}")
                 for k in range(BC)]
        for k in range(BC):
            wdma(wu_sb[k][:], wu_d.bitcast(F32R)[bass.ts(k, P), :])
        if include_bd:
            bd_sb = wpool.tile([P, BC], F32, tag="bd")
            nc.sync.dma_start(bd_sb[:], bd_d)
        if include_bu:
            bu_sb = wpool.tile([E, D], F32R, tag="bu")
            nc.sync.dma_start(bu_sb[:], bu_d.bitcast(F32R))

        for rep in range(reps):
            g2ts = []
            # --- phase 1: logits + gating for every block ---
            for blk in range(NLB):
                cols = bass.ts(blk, LBLK)
                lt_ps = lt_ps_pool.tile([E, LBLK], F32, tag="ltps")
                n_mm = 3 * KC
                i = 0
                for c in range(KC):
                    for lhsT, rhs in ((wgh_sb[:, c, :], xht[c][:, cols]),
                                      (wgl_sb[:, c, :], xht[c][:, cols]),
                                      (wgh_sb[:, c, :], xlt[c][:, cols])):
                        nc.tensor.matmul(lt_ps[:], lhsT, rhs,
                                         start=(i == 0), stop=(i == n_mm - 1))
                        i += 1
                lt_sb = gpool.tile([E, LBLK], F32, tag="ltsb")
                nc.scalar.copy(lt_sb[:], lt_ps[:])

                g2t_blk = gpool.tile([E, LBLK], F32R, tag="g2t",
                                     name=f"g2t{blk}")
                for bo in range(TPB):
                    small_ps = lb_ps_pool.tile([P, E + P], F32, tag="lbsmall")
                    lb_ps = small_ps[:, 0:E]
                    g2t_ps = small_ps[0:E, E:E + P].bitcast(F32R)
                    nc.tensor.transpose(lb_ps, lt_sb[:, bass.ts(bo, P)],
                                        ident[0:E, 0:E])
                    l_sb = gpool.tile([P, E], F32, tag="lsb")
                    nc.scalar.copy(l_sb[:], lb_ps)

                    m1 = gpool.tile([P, 1], F32, tag="m1")
                    nc.vector.tensor_reduce(m1[:], l_sb[:], AX.X, AL.max)
                    m1n = gpool.tile([P, 1], F32, tag="m1n")
                    nc.vector.tensor_scalar_mul(m1n[:], m1[:], -1.0)
                    mask1 = gpool.tile([P, E], F32, tag="mask1")
                    nc.vector.tensor_scalar(mask1[:], l_sb[:], m1[:], None,
                                            op0=AL.is_ge)
                    lm = gpool.tile([P, E], F32, tag="lm")
                    nc.vector.scalar_tensor_tensor(
                        lm[:], mask1[:], -1e30, l_sb[:],
                        op0=AL.mult, op1=AL.add)
                    m2 = gpool.tile([P, 1], F32, tag="m2")
                    nc.vector.tensor_reduce(m2[:], lm[:], AX.X, AL.max)
                    e2 = gpool.tile([P, 1], F32, tag="e2")
                    nc.scalar.activation(e2[:], m2[:], ACTF.Exp, bias=m1n[:])
                    d2 = gpool.tile([P, 1], F32, tag="d2")
                    nc.scalar.activation(d2[:], e2[:], ACTF.Copy,
                                         bias=1.0 / SCALE, scale=1.0 / SCALE)
                    rh = gpool.tile([P, 1], F32, tag="rh")
                    nc.vector.reciprocal(rh[:], d2[:])
                    expl = gpool.tile([P, E], F32, tag="expl")
                    nc.scalar.activation(expl[:], l_sb[:], ACTF.Exp,
                                         bias=m1n[:])
                    mask2 = gpool.tile([P, E], F32, tag="mask2")
                    nc.vector.tensor_scalar(mask2[:], l_sb[:], m2[:], None,
                                            op0=AL.is_ge)
                    g2 = gpool.tile([P, E], F32, tag="g2")
                    nc.vector.scalar_tensor_tensor(
                        g2[:], expl[:], rh[:], mask2[:],
                        op0=AL.mult, op1=AL.mult)
                    # transpose gates to [8, tok] (f32r)
                    g2r = gpool.tile([P, E], F32R, tag="g2r")
                    nc.vector.tensor_copy(g2r[:], g2[:])
                    nc.tensor.transpose(g2t_ps, g2r[:], ident_r[:])
                    nc.scalar.copy(g2t_blk[:, bass.ts(bo, P)], g2t_ps)
                g2ts.append(g2t_blk)

            # --- phase 2: expert compute per block ---
            for blk in range(NLB):
                cols = bass.ts(blk, LBLK)
                g2t_blk = g2ts[blk]

                hgs = []
                for k in range(BC):
                    ht_ps = ht_ps_pool.tile([P, LBLK], F32, tag="htps")
                    for c in range(KC):
                        nc.tensor.matmul(
                            ht_ps[:], wd_sb[c][:, bass.ts(k, P)],
                            xtr[c][:, cols],
                            start=(c == 0), stop=(c == KC - 1))
                    r_k = hgpool.tile([P, LBLK], F32, tag="relu")
                    if include_bd:
                        nc.scalar.activation(r_k[:], ht_ps[:], ACTF.Relu,
                                             bias=bd_sb[:, k:k + 1])
                    else:
                        nc.scalar.activation(r_k[:], ht_ps[:], ACTF.Relu)
                    # GB = Eblk-chunk^T @ g2T : per-token gate per partition
                    gb_ps = gb_ps_pool.tile([P, LBLK], F32R, tag="gbps")
                    nc.tensor.matmul(gb_ps[:].bitcast(F32),
                                     eblk[:, bass.ts(k, P)], g2t_blk[:],
                                     start=True, stop=True)
                    hg_k = hgpool.tile([P, LBLK], F32R, tag="hg",
                                       name=f"hg{blk}_{k}")
                    nc.vector.tensor_tensor(hg_k[:], r_k[:],
                                            gb_ps[:].bitcast(F32),
                                            op=AL.mult)
                    hgs.append(hg_k)

                # step B: out tiles = HG @ Wu_flat (+ g2 @ bu)
                for bo in range(TPB):
                    t = blk * TPB + bo
                    rows = bass.ts(t, P)
                    tok = bass.ts(bo, P)
                    for h in range(2):
                        o_ps = o_ps_pool.tile([P, 512], F32, tag="ops")
                        n_b = BC + (1 if include_bu else 0)
                        for k in range(BC):
                            nc.tensor.matmul(
                                o_ps[:], hgs[k][:, tok],
                                wu_sb[k][:, bass.ts(h, 512)],
                                start=(k == 0), stop=(k == n_b - 1))
                        if include_bu:
                            nc.tensor.matmul(o_ps[:], g2t_blk[:, tok],
                                             bu_sb[:, bass.ts(h, 512)],
                                             start=False, stop=True)
                        o_sb = opool.tile([P, 512], F32, tag="osb")
                        if h == 0:
                            nc.vector.tensor_copy(o_sb[:], o_ps[:])
                        else:
                            nc.scalar.copy(o_sb[:], o_ps[:])
                        nc.scalar.dma_start(out_d[rows, bass.ts(h, 512)],
                                            o_sb[:])

    nc.compile()
    _BUILD_CACHE[key] = nc
    return nc


def _split_bf16(a):
    hi = a.astype(ml_dtypes.bfloat16)
    lo = (a - hi.astype(np.float32)).astype(ml_dtypes.bfloat16)
    return hi, lo


def kernel(x, w_gate, w_noise, Wd, bd, Wu, bu, reps: int = 1):
    x = np.ascontiguousarray(np.asarray(x, dtype=np.float32))
    assert x.shape == (B_DIM, S_DIM, D), x.shape
    wg = np.ascontiguousarray(np.asarray(w_gate, dtype=np.float32))
    Wd = np.asarray(Wd, dtype=np.float32)
    Wu = np.asarray(Wu, dtype=np.float32)
    bd = np.asarray(bd, dtype=np.float32)
    bu = np.asarray(bu, dtype=np.float32)

    include_bd = bool(np.any(bd))
    include_bu = bool(np.any(bu))
    nc = _build(include_bd, include_bu, reps)

    xf = x.reshape(T, D)
    xh, xl = _split_bf16(xf)
    xht_full = np.ascontiguousarray(xh.T)   # [D, T]
    xlt_full = np.ascontiguousarray(xl.T)
    wgh, wgl = _split_bf16(wg)
    wd_all = np.ascontiguousarray(
        Wd.transpose(1, 0, 2).reshape(D, EB))          # [D, E*BK]
    wu_flat = np.ascontiguousarray(Wu.reshape(EB, D))  # [E*BK, D]
    ident = np.eye(P, dtype=np.float32)
    eblk = np.kron(np.eye(E, dtype=np.float32),
                   np.ones((1, BK), dtype=np.float32))  # [E, EB]

    shared = dict(wd=wd_all, wu=wu_flat, wgh=wgh, wgl=wgl, ident=ident,
                  eblk=eblk)
    if include_bd:
        # [P, BC] partition-major per chunk: bd_sb[p, k] = bd_flat[128k+p]
        shared["bd"] = np.ascontiguousarray(
            bd.reshape(EB)[np.arange(P)[:, None] + P * np.arange(BC)[None]])
    if include_bu:
        shared["bu"] = np.ascontiguousarray(bu)

    in_maps = []
    for c in range(N_CORES):
        sl = slice(c * TC, (c + 1) * TC)
        in_maps.append(dict(xh=np.ascontiguousarray(xht_full[:, sl]),
                            xl=np.ascontiguousarray(xlt_full[:, sl]),
                            **shared))
    kernel.last_in_maps = in_maps
    res = run_bass_kernel_spmd(nc, in_maps, core_ids=list(range(N_CORES)))
    out = np.concatenate([res.results[c]["out"] for c in range(N_CORES)], axis=0)
    return out.reshape(B_DIM, S_DIM, D).astype(np.float32)

